# revision 10
# baseline (speedup 1.0000x reference)
"""Trainium2 Bass kernel for nn_Attention_79070347919638 (gnn_message_passing).

Point-cloud ball-query attention, data-parallel over batch: 16 batches -> 8
NeuronCores x 2 batches each. Per core: LayerNorm+QKV on PE, ball query via a
split-bf16 exact pairwise quadratic-form matmul (16-term hi/lo decomposition,
bf16 operands, fp32 PSUM accumulate) + top-8 smallest-index extraction with
InstMax on an index-encoded value, neighbor gather via one batched indirect
DMA per tile from a DRAM-staged [k|v|xyz] row table, per-point attention on
DVE, output projection + GELU + residual on PE/ACT/DVE.
"""
import sys
import numpy as np

sys.path.insert(0, "/opt/trn_rl_repo")

B, N, D = 16, 2048, 256
H, DH, KNB = 8, 64, 8
I = H * DH  # 512
R2 = 0.09
EPS = 1e-5
NCORES = 8
NB = B // NCORES  # batches per core
P = 128
NT = N // P  # n-tiles per batch
ROW = 1152  # gathered row: k(512) | v(512) | xyz(3) | pad -> 2304B (%256==0 for dma_gather)
BIG_C = 2048.0  # index encoding: val = BIG_C - m for in-radius m
QD = 16  # split-bf16 quadratic form contraction size



def _ap(view, dims):
    """Build an AP from a view's tensor with explicit [step,count] dims."""
    import concourse.bass as bass
    return bass.AP(tensor=view.tensor, offset=view.offset, ap=list(dims))


def _bcast_mid(view3, n):
    """[p, 1, x] view -> [p, n(stride0), x]."""
    return _ap(view3, [view3.ap[0], [0, n], view3.ap[2]])


def _bcast_last(view, n):
    """[p, ...] view -> same dims + [0, n] appended."""
    return _ap(view, list(view.ap) + [[0, n]])


def _build_nc():
    import concourse.bass as bass
    import concourse.bacc as bacc
    import concourse.mybir as mybir
    import concourse.tile as tile
    from concourse.masks import make_identity
    from contextlib import ExitStack

    dt = mybir.dt
    Alu = mybir.AluOpType
    Act = mybir.ActivationFunctionType
    Axis = mybir.AxisListType

    nc = bacc.Bacc("TRN2", target_bir_lowering=False, debug=False,
                   num_devices=NCORES)

    xyzs_d = nc.dram_tensor("xyzs", [NB, N, 3], dt.float32, kind="ExternalInput").ap()
    feat_d = nc.dram_tensor("feature", [NB, N, D], dt.float32, kind="ExternalInput").ap()
    lng_d = nc.dram_tensor("ln_g", [D], dt.float32, kind="ExternalInput").ap()
    lnb_d = nc.dram_tensor("ln_b", [D], dt.float32, kind="ExternalInput").ap()
    wqkv_d = nc.dram_tensor("w_qkv", [D, 3 * I], dt.float32, kind="ExternalInput").ap()
    wsp_d = nc.dram_tensor("w_sp", [3, DH], dt.float32, kind="ExternalInput").ap()
    wout_d = nc.dram_tensor("w_out", [I, D], dt.float32, kind="ExternalInput").ap()
    bout_d = nc.dram_tensor("b_out", [D], dt.float32, kind="ExternalInput").ap()
    out_d = nc.dram_tensor("out", [NB, N, D], dt.float32, kind="ExternalOutput").ap()

    kv_d = [nc.dram_tensor(f"kvrows{b}", [N, ROW], dt.bfloat16).ap()
            for b in range(NB)]

    ctx = ExitStack()
    with tile.TileContext(nc) as tc, ctx:
        cpool = ctx.enter_context(tc.tile_pool(name="const", bufs=1))
        sb = ctx.enter_context(tc.tile_pool(name="sb", bufs=2))
        sb3 = ctx.enter_context(tc.tile_pool(name="sb3", bufs=4))
        w1 = ctx.enter_context(tc.tile_pool(name="w1", bufs=1))
        sb2a = ctx.enter_context(tc.tile_pool(name="sb2a", bufs=4))
        sbg = ctx.enter_context(tc.tile_pool(name="sbg", bufs=2))
        ps_tr = ctx.enter_context(tc.tile_pool(name="ps_tr", bufs=2, space="PSUM"))
        ps_po = ctx.enter_context(tc.tile_pool(name="ps_po", bufs=2, space="PSUM"))
        ps_qkv = ctx.enter_context(tc.tile_pool(name="ps_qkv", bufs=2, space="PSUM"))
        ps_d2 = ctx.enter_context(tc.tile_pool(name="ps_d2", bufs=1, space="PSUM"))

        # ================= one-time constants =================
        ident = cpool.tile([P, P], dt.bfloat16)
        make_identity(nc, ident[:])

        iota_h = cpool.tile([P, N], dt.float16)
        nc.gpsimd.iota(iota_h[:], pattern=[[-1, N]], base=int(BIG_C),
                       channel_multiplier=0,
                       allow_small_or_imprecise_dtypes=True)

        # ln_g-scaled w_qkv (bf16), two K-chunks along free: [128, 2*1536]
        wq_sb = cpool.tile([P, 2 * 3 * I], dt.bfloat16)
        g_col = cpool.tile([P, 2], dt.float32)
        nc.sync.dma_start(g_col[:], lng_d.rearrange("(c p) -> p c", p=P))
        for c in range(2):
            wtmp = w1.tile([P, 3 * I], dt.float32, tag="wtmp")
            nc.sync.dma_start(wtmp[:], wqkv_d[c * P:(c + 1) * P, :])
            nc.vector.tensor_scalar_mul(wq_sb[:, c * 3 * I:(c + 1) * 3 * I],
                                        wtmp[:], g_col[:, c:c + 1])
        # bw = ln_b @ w_qkv  [1, 1536]
        b_col = cpool.tile([P, 2], dt.float32)
        nc.sync.dma_start(b_col[:], lnb_d.rearrange("(c p) -> p c", p=P))
        b_colb = cpool.tile([P, 2], dt.bfloat16)
        nc.vector.tensor_copy(b_colb[:], b_col[:])
        bw_rowb = cpool.tile([1, 3 * I], dt.bfloat16)
        for ch in range(3):
            bw_ps = ps_tr.tile([1, I], dt.float32, tag="ptr")
            for c in range(2):
                nc.tensor.matmul(bw_ps[:1, :], lhsT=b_colb[:, c:c + 1],
                                 rhs=wq_sb[:, c * 3 * I + ch * I:
                                           c * 3 * I + (ch + 1) * I],
                                 start=(c == 0), stop=(c == 1))
            nc.scalar.copy(bw_rowb[:1, ch * I:(ch + 1) * I], bw_ps[:1, :])
        ones1 = cpool.tile([1, P], dt.bfloat16)
        nc.vector.memset(ones1[:1, :], 1.0)

        wout_sb = cpool.tile([P, 4 * D], dt.bfloat16)
        for c in range(4):
            wotmp = w1.tile([P, D], dt.float32, tag="wotmp")
            nc.sync.dma_start(wotmp[:], wout_d[c * P:(c + 1) * P, :])
            nc.vector.tensor_copy(wout_sb[:, c * D:(c + 1) * D], wotmp[:])
        bout_row = cpool.tile([1, D], dt.bfloat16)
        btmp = cpool.tile([1, D], dt.float32)
        nc.sync.dma_start(btmp[:1, :], bout_d[None, :])
        nc.vector.tensor_copy(bout_row[:1, :], btmp[:1, :])

        # WSPOUT = blockdiag_h(w_sp) @ w_out : [24, 256] bf16
        identf = cpool.tile([P, P], dt.float32)
        make_identity(nc, identf[:])
        wsp_sb = cpool.tile([3, DH], dt.float32)
        nc.sync.dma_start(wsp_sb[:3, :], wsp_d[:, :])
        wspT_ps = ps_tr.tile([DH, 3], dt.float32, tag="ptr")
        nc.tensor.transpose(wspT_ps[:DH, :3], wsp_sb[:3, :], identf[:3, :3])
        wspT = cpool.tile([DH, 3], dt.bfloat16)
        nc.scalar.copy(wspT[:DH, :3], wspT_ps[:DH, :3])
        wspbd = cpool.tile([P, 4 * 24], dt.bfloat16)
        nc.vector.memset(wspbd[:], 0.0)
        for h in range(H):
            c, off = divmod(h * DH, P)
            nc.vector.tensor_copy(
                wspbd[off:off + DH, c * 24 + h * 3:c * 24 + h * 3 + 3],
                wspT[:DH, :3])
        wspout_ps = ps_tr.tile([24, D], dt.float32, tag="ptr")
        for c in range(4):
            nc.tensor.matmul(wspout_ps[:24, :], lhsT=wspbd[:, c * 24:(c + 1) * 24],
                             rhs=wout_sb[:, c * D:(c + 1) * D],
                             start=(c == 0), stop=(c == 3))
        wspout = cpool.tile([24, D], dt.bfloat16)
        nc.scalar.copy(wspout[:24, :], wspout_ps[:24, :])

        # ---- constants for the wrapped int16 gather-index build ----
        # h16t[r, p] = (p % 16 == r); g16t[g, p] = (p // 16 == g)
        h16t = cpool.tile([16, P], dt.float32)
        nc.gpsimd.iota(h16t[:16, :], pattern=[[0, 8], [-1, 16]], base=16,
                       channel_multiplier=1,
                       allow_small_or_imprecise_dtypes=True)
        nc.vector.tensor_scalar(h16t[:16, :], h16t[:16, :], 16.0, None,
                                op0=Alu.is_equal)
        g16t = cpool.tile([8, P], dt.float32)
        nc.gpsimd.iota(g16t[:8, :], pattern=[[-1, 8], [0, 16]], base=8,
                       channel_multiplier=1,
                       allow_small_or_imprecise_dtypes=True)
        nc.vector.tensor_scalar(g16t[:8, :], g16t[:8, :], 8.0, None,
                                op0=Alu.is_equal)
        # s128[p, j] = (p % 16 == j % 16) = h16 @ h16t
        s128_ps = ps_tr.tile([P, P], dt.float32, tag="ptr")
        nc.tensor.matmul(s128_ps[:], lhsT=h16t[:16, :], rhs=h16t[:16, :],
                         start=True, stop=True)
        s128 = cpool.tile([P, P], dt.float32)
        nc.scalar.copy(s128[:], s128_ps[:])
        # g16[p, g] = (p // 16 == g)
        g16_ps = ps_tr.tile([P, 8], dt.float32, tag="ptr")
        nc.tensor.transpose(g16_ps[:, :8], g16t[:8, :], identf[:8, :8])
        g16 = cpool.tile([P, 8], dt.float32)
        nc.scalar.copy(g16[:], g16_ps[:, :8])

        # ========== phase A / phase B as per-tile emitters ==========
        def phase_a_prologue(b):
            xyz_t = sb.tile([P, NT * 3], dt.float32, tag="xyz")
            nc.sync.dma_start(
                xyz_t[:].rearrange("p (t c) -> p t c", c=3),
                xyzs_d[b].rearrange("(t p) c -> p t c", p=P))
            sq = sb.tile([P, NT * 3], dt.float32, tag="sq")
            nc.vector.tensor_mul(sq[:], xyz_t[:], xyz_t[:])
            x2 = sb.tile([P, NT], dt.float32, tag="x2")
            nc.vector.tensor_reduce(
                x2[:], sq[:].rearrange("p (t c) -> p t c", c=3),
                axis=Axis.X, op=Alu.add)

            # ---- split-bf16 components ----
            s_f = sb.tile([P, NT], dt.float32, tag="s_f")
            nc.vector.tensor_scalar(s_f[:], x2[:], -1.0, float(R2),
                                    op0=Alu.mult, op1=Alu.add)
            sh_b = sb.tile([P, NT], dt.bfloat16, tag="sh_b")
            nc.vector.tensor_copy(sh_b[:], s_f[:])
            sh_f = sb.tile([P, NT], dt.float32, tag="sh_f")
            nc.vector.tensor_copy(sh_f[:], sh_b[:])
            sl_b = sb.tile([P, NT], dt.bfloat16, tag="sl_b")
            nc.vector.tensor_sub(s_f[:], s_f[:], sh_f[:])
            nc.vector.tensor_copy(sl_b[:], s_f[:])
            ny2 = sb.tile([P, NT], dt.float32, tag="ny2")
            nc.vector.tensor_scalar_mul(ny2[:], x2[:], -1.0)
            y2h_b = sb.tile([P, NT], dt.bfloat16, tag="y2h_b")
            nc.vector.tensor_copy(y2h_b[:], ny2[:])
            y2h_f = sb.tile([P, NT], dt.float32, tag="y2h_f")
            nc.vector.tensor_copy(y2h_f[:], y2h_b[:])
            y2l_b = sb.tile([P, NT], dt.bfloat16, tag="y2l_b")
            nc.vector.tensor_sub(ny2[:], ny2[:], y2h_f[:])
            nc.vector.tensor_copy(y2l_b[:], ny2[:])
            xh_b = sb.tile([P, NT * 3], dt.bfloat16, tag="xh_b")
            nc.vector.tensor_copy(xh_b[:], xyz_t[:])
            xh_f = sb.tile([P, NT * 3], dt.float32, tag="xh_f")
            nc.vector.tensor_copy(xh_f[:], xh_b[:])
            xl_f = sb.tile([P, NT * 3], dt.float32, tag="xl_f")
            nc.vector.tensor_sub(xl_f[:], xyz_t[:], xh_f[:])
            xl_b = sb.tile([P, NT * 3], dt.bfloat16, tag="xl_b")
            nc.vector.tensor_copy(xl_b[:], xl_f[:])

            palla = sb.tile([P, NT * QD], dt.bfloat16, tag="palla")
            pallb = sb.tile([P, NT * QD], dt.bfloat16, tag="pallb")
            pva = palla[:].rearrange("p (t q) -> p t q", q=QD)
            pvb = pallb[:].rearrange("p (t q) -> p t q", q=QD)
            xhv = xh_b[:].rearrange("p (t c) -> p t c", c=3)
            xlv = xl_b[:].rearrange("p (t c) -> p t c", c=3)
            nc.vector.tensor_copy(pva[:, :, 0], sh_b[:])
            nc.vector.tensor_copy(pva[:, :, 1], sl_b[:])
            nc.vector.memset(pva[:, :, 2:4], 1.0)
            for c in range(3):
                nc.vector.tensor_scalar_mul(pva[:, :, 4 + 4 * c], xhv[:, :, c], 2.0)
                nc.vector.tensor_copy(pva[:, :, 5 + 4 * c], pva[:, :, 4 + 4 * c])
                nc.vector.tensor_scalar_mul(pva[:, :, 6 + 4 * c], xlv[:, :, c], 2.0)
                nc.vector.tensor_copy(pva[:, :, 7 + 4 * c], pva[:, :, 6 + 4 * c])
            nc.vector.memset(pvb[:, :, 0:2], 1.0)
            nc.vector.tensor_copy(pvb[:, :, 2], y2h_b[:])
            nc.vector.tensor_copy(pvb[:, :, 3], y2l_b[:])
            for c in range(3):
                nc.vector.tensor_copy(pvb[:, :, 4 + 4 * c], xhv[:, :, c])
                nc.vector.tensor_copy(pvb[:, :, 5 + 4 * c], xlv[:, :, c])
                nc.vector.tensor_copy(pvb[:, :, 6 + 4 * c], xhv[:, :, c])
                nc.vector.tensor_copy(pvb[:, :, 7 + 4 * c], xlv[:, :, c])

            a4 = sb.tile([P, N], dt.bfloat16, tag="a4")
            b4 = sb.tile([P, N], dt.bfloat16, tag="b4")
            for t in range(NT):
                s_ = slice(t * P, (t + 1) * P)
                for (pt, dst) in ((palla, a4), (pallb, b4)):
                    trp16 = ps_tr.tile([QD, P], dt.bfloat16, tag="ptr")
                    nc.tensor.transpose(trp16[:QD, :],
                                        pt[:, t * QD:(t + 1) * QD], ident[:])
                    nc.scalar.copy(dst[0:QD, s_], trp16[:QD, :])
            for st in (32, 64, 96):
                nc.vector.tensor_copy(a4[st:st + QD, :], a4[0:QD, :])
                nc.vector.tensor_copy(b4[st:st + QD, :], b4[0:QD, :])

            q_sb = sb.tile([P, NT * I], dt.bfloat16, tag="q_sb")
            spread = sb.tile([P, NT * 64], dt.float32, tag="spread")
            idx16w = sb.tile([P, NT * 64], dt.int16, tag="idx16w")
            return dict(xyz_t=xyz_t, a4=a4, b4=b4, q_sb=q_sb,
                        spread=spread, idx16w=idx16w)

        def phase_a_tile(b, pa, t):
            xyz_t = pa["xyz_t"]; a4 = pa["a4"]; b4 = pa["b4"]
            q_sb = pa["q_sb"]; spread = pa["spread"]
            ftile = sb3.tile([P, D], dt.float32, tag="ftile")
            nc.sync.dma_start(ftile[:], feat_d[b, t * P:(t + 1) * P, :])
            mean = sb3.tile([P, 1], dt.float32, tag="mean")
            nc.vector.tensor_reduce(mean[:], ftile[:], axis=Axis.X, op=Alu.add)
            nc.vector.tensor_scalar_mul(mean[:], mean[:], 1.0 / D)
            var = sb3.tile([P, 1], dt.float32, tag="var")
            sttd = sb3.tile([P, D], dt.float32, tag="ftile")
            nc.vector.scalar_tensor_tensor(
                sttd[:], in0=ftile[:], scalar=mean[:, :1], in1=ftile[:],
                op0=Alu.subtract, op1=Alu.mult, accum_out=var[:, :1])
            rstd = sb3.tile([P, 1], dt.float32, tag="rstd")
            nc.vector.tensor_scalar(rstd[:], var[:], 1.0 / D, EPS,
                                    op0=Alu.mult, op1=Alu.add)
            nc.vector.reciprocal(rstd[:], rstd[:])
            nc.scalar.sqrt(rstd[:], rstd[:])
            zn = sb3.tile([P, D], dt.bfloat16, tag="zn")
            nc.vector.tensor_scalar(zn[:], ftile[:], mean[:, :1], rstd[:, :1],
                                    op0=Alu.subtract, op1=Alu.mult)
            znT = sb3.tile([P, 2 * P], dt.bfloat16, tag="znT")
            znT_ps = ps_tr.tile([P, 2 * P], dt.bfloat16, tag="ptr")
            for c in range(2):
                nc.tensor.transpose(znT_ps[:, c * P:(c + 1) * P],
                                    zn[:, c * P:(c + 1) * P], ident[:])
            nc.scalar.copy(znT[:], znT_ps[:])
            kv_sb = sb3.tile([P, ROW], dt.bfloat16, tag="kv_sb")
            for ch in range(3):
                qkv_ps = ps_qkv.tile([P, I], dt.float32, tag="qkv")
                for c in range(2):
                    nc.tensor.matmul(
                        qkv_ps[:], lhsT=znT[:, c * P:(c + 1) * P],
                        rhs=wq_sb[:, c * 3 * I + ch * I:
                                  c * 3 * I + (ch + 1) * I],
                        start=(c == 0), stop=False)
                nc.tensor.matmul(
                    qkv_ps[:], lhsT=ones1[:1, :],
                    rhs=bw_rowb[:1, ch * I:(ch + 1) * I],
                    start=False, stop=True)
                if ch == 0:
                    nc.scalar.copy(q_sb[:, t * I:(t + 1) * I], qkv_ps[:])
                else:
                    nc.scalar.copy(kv_sb[:, (ch - 1) * I:ch * I], qkv_ps[:])
            nc.vector.tensor_copy(kv_sb[:, 2 * I:2 * I + 3],
                                  xyz_t[:, t * 3:(t + 1) * 3])
            nc.sync.dma_start(kv_d[b][t * P:(t + 1) * P, :], kv_sb[:])

            # ball query for this tile: d2aug via split-bf16 matmul
            sgn = sb.tile([P, N], dt.float16, tag="sgn")
            for half in range(2):
                d2ps = ps_d2.tile([P, N // 2], dt.float32, tag="d2")
                for j in range(2):
                    mi = half * 2 + j
                    st = 32 * mi
                    nc.tensor.matmul(
                        d2ps[:, j * 512:(j + 1) * 512],
                        lhsT=a4[st:st + QD, t * P:(t + 1) * P],
                        rhs=b4[st:st + QD, mi * 512:(mi + 1) * 512],
                        start=True, stop=True,
                        tile_position=(st, 0))
                nc.scalar.sign(sgn[:, half * (N // 2):(half + 1) * (N // 2)],
                               d2ps[:])
            nc.vector.tensor_mul(sgn[:], sgn[:], iota_h[:])
            v8 = sb.tile([P, 8], dt.float16, tag="v8")
            nc.vector.max(out=v8[:], in_=sgn[:])
            idxf = sb.tile([P, 8], dt.float32, tag="idxf")
            nc.vector.tensor_scalar(idxf[:], v8[:], -1.0, float(BIG_C),
                                    op0=Alu.mult, op1=Alu.add)
            pred = sb.tile([P, 8], dt.uint8, tag="pred")
            nc.vector.tensor_scalar(pred[:], v8[:], 0.0, None, op0=Alu.is_gt)
            idxf2 = sb.tile([P, 8], dt.float32, tag="idxf2")
            nc.vector.select(idxf2[:], pred[:], idxf[:],
                             _ap(idxf[:, 0:1], [idxf[:, 0:1].ap[0], [0, 8]]))
            # spread[p, (k, pg)] = idxf2[p, k] * (p // 16 == pg)
            sp = spread[:, t * 64:(t + 1) * 64]
            nc.vector.tensor_mul(
                sp.rearrange("p (k g) -> p k g", k=8),
                _bcast_last(idxf2[:], 8),
                _ap(g16[:], [g16[:].ap[0], [0, 8], [1, 8]]))

        def phase_b_tile(b, pa, t):
            xyz_t = pa["xyz_t"]; q_sb = pa["q_sb"]; idx16w = pa["idx16w"]
            kvg = sbg.tile([P, 8 * ROW], dt.bfloat16, tag="kvg")
            nc.gpsimd.dma_gather(
                out_ap=kvg[:].rearrange("p (k r) -> p k r", k=8),
                in_ap=kv_d[b][:, :],
                idxs_ap=idx16w[:, t * 64:(t + 1) * 64],
                num_idxs=1024,
                num_idxs_reg=1024,
                elem_size=ROW,
                queue_num=0,
            )
            kview = kvg[:].rearrange("p (k r) -> p k r", k=8)
            qv = q_sb[:, t * I:(t + 1) * I].rearrange("p (o i) -> p o i", o=1)
            wq = sb2a.tile([P, 8 * I], dt.bfloat16, tag="wq")
            nc.vector.tensor_mul(
                wq[:].rearrange("p (k i) -> p k i", k=8),
                kview[:, :, 0:I], _bcast_mid(qv, 8))
            width = DH
            while width > 1:
                half = width // 2
                a = wq[:].rearrange("p (kh w) -> p kh w", w=DH)
                nc.vector.tensor_add(
                    a[:, :, 0:half], a[:, :, 0:half], a[:, :, half:width])
                width = half
            wexp = sb3.tile([P, 64], dt.bfloat16, tag="wexp")
            nc.scalar.activation(
                wexp[:],
                _ap(wq[:], [wq[:].ap[0], [DH, 64]]),
                Act.Exp, scale=float(DH ** -0.5))
            z = sb3.tile([P, 8], dt.float32, tag="z")
            nc.vector.tensor_reduce(
                z[:],
                _ap(wexp[:], [wexp[:].ap[0], [1, 8], [8, 8]]),
                axis=Axis.X, op=Alu.add)
            nc.vector.reciprocal(z[:], z[:])
            zb = sb3.tile([P, 8], dt.bfloat16, tag="zb")
            nc.vector.tensor_copy(zb[:], z[:])
            attn = sb3.tile([P, 64], dt.bfloat16, tag="attn")  # [k,h]
            we2 = wexp[:].rearrange("p (k h) -> p k h", k=8)
            zv = zb[:].rearrange("p (o h) -> p o h", o=1)
            nc.vector.tensor_mul(
                attn[:].rearrange("p (k h) -> p k h", k=8),
                we2, _bcast_mid(zv, 8))
            att2 = sb3.tile([P, P], dt.bfloat16, tag="att2")
            nc.vector.tensor_copy(
                att2[:].rearrange("p (j e) -> p j e", e=2),
                _bcast_last(attn[:].rearrange("p (o j) -> p o j", o=1)[:, 0, :], 2))
            wv = sb2a.tile([P, 8 * I], dt.bfloat16, tag="wq")
            a2 = att2[:]
            nc.vector.tensor_mul(
                _ap(wv[:], [wv[:].ap[0], [512, 8], [64, 8], [2, 32], [1, 2]]),
                _ap(kvg[:, I:I + 1],
                    [kvg[:].ap[0], [ROW, 8], [64, 8], [2, 32], [1, 2]]),
                _ap(a2, [a2.ap[0], [16, 8], [2, 8], [0, 32], [1, 2]]))
            wv2 = wv[:].rearrange("p (k i) -> p k i", k=8)
            nc.vector.tensor_add(wv2[:, 0:4, :], wv2[:, 0:4, :], wv2[:, 4:8, :])
            nc.vector.tensor_add(wv2[:, 0:2, :], wv2[:, 0:2, :], wv2[:, 2:4, :])
            ao = sb3.tile([P, I], dt.bfloat16, tag="ao")
            nc.vector.tensor_add(ao[:].rearrange("p (o i) -> p o i", o=1),
                                 wv2[:, 0:1, :], wv2[:, 1:2, :])
            disp = sb3.tile([P, 24], dt.bfloat16, tag="disp")  # [k,c]
            xv = xyz_t[:, t * 3:(t + 1) * 3].rearrange("p (o c) -> p o c", o=1)
            nc.vector.tensor_sub(
                disp[:].rearrange("p (k c) -> p k c", k=8),
                kview[:, :, 2 * I:2 * I + 3], _bcast_mid(xv, 8))
            dprod = sb3.tile([P, H * 8 * 3], dt.bfloat16, tag="dprod")
            dp3 = dprod[:].rearrange("p (h k c) -> p h k c", h=H, k=8)
            dview = disp[:].rearrange("p (k c) -> p k c", k=8)
            ahk = attn[:].rearrange("p (k h) -> p h k", k=8)
            nc.vector.tensor_mul(
                dp3,
                _ap(dview, [dview.ap[0], [0, H], dview.ap[1], dview.ap[2]]),
                _bcast_last(ahk, 3))
            TT = nc.vector.tensor_tensor
            TT(dp3[:, :, 0:4, :], dp3[:, :, 0:4, :], dp3[:, :, 4:8, :],
               op=Alu.max)
            TT(dp3[:, :, 0:2, :], dp3[:, :, 0:2, :], dp3[:, :, 2:4, :],
               op=Alu.max)
            dis = sb3.tile([P, 24], dt.bfloat16, tag="dis")  # [h,c]
            TT(dis[:].rearrange("p (h o c) -> p h o c", o=1, c=3),
               dp3[:, :, 0:1, :], dp3[:, :, 1:2, :], op=Alu.max)
            aot = sb3.tile([P, 4 * P], dt.bfloat16, tag="aot")
            aot_ps = ps_tr.tile([P, 4 * P], dt.bfloat16, tag="ptr")
            for c in range(4):
                nc.tensor.transpose(aot_ps[:, c * P:(c + 1) * P],
                                    ao[:, c * P:(c + 1) * P], ident[:])
            nc.scalar.copy(aot[:], aot_ps[:])
            dist = sb3.tile([24, P], dt.bfloat16, tag="dist")
            trp = ps_tr.tile([P, P], dt.bfloat16, tag="ptr")
            nc.tensor.transpose(trp[:24, :], dis[:, :24], ident[:])
            nc.scalar.copy(dist[:24, :], trp[:24, :])
            po = ps_po.tile([P, D], dt.float32, tag="po")
            for c in range(4):
                nc.tensor.matmul(po[:], lhsT=aot[:, c * P:(c + 1) * P],
                                 rhs=wout_sb[:, c * D:(c + 1) * D],
                                 start=(c == 0), stop=False)
            nc.tensor.matmul(po[:], lhsT=dist[:24, :], rhs=wspout[:24, :],
                             start=False, stop=False)
            nc.tensor.matmul(po[:], lhsT=ones1[:1, :], rhs=bout_row[:1, :],
                             start=False, stop=True)
            gel = sb3.tile([P, D], dt.float32, tag="gel")
            nc.scalar.activation(gel[:], po[:], Act.Gelu)
            f2 = sb3.tile([P, D], dt.float32, tag="ftile")
            nc.sync.dma_start(f2[:], feat_d[b, t * P:(t + 1) * P, :])
            outt = sb3.tile([P, D], dt.float32, tag="outt")
            nc.vector.tensor_add(outt[:], gel[:], f2[:])
            nc.sync.dma_start(out_d[b, t * P:(t + 1) * P, :], outt[:])

        # ===== schedule: A(b0) | A(b1)+B(b0) interleaved | B(b1) =====
        def finish_idx(pa):
            # idx16w[p, m (global)] = sum_q s128[q, p] * spread[q, m]
            for c2 in range(2):
                wps = ps_tr.tile([P, 512], dt.float32, tag="ptr")
                nc.tensor.matmul(wps[:], lhsT=s128[:],
                                 rhs=pa["spread"][:, c2 * 512:(c2 + 1) * 512],
                                 start=True, stop=True)
                nc.vector.tensor_copy(pa["idx16w"][:, c2 * 512:(c2 + 1) * 512],
                                      wps[:])

        pa0 = phase_a_prologue(0)
        for t in range(NT):
            phase_a_tile(0, pa0, t)
        finish_idx(pa0)
        pa1 = phase_a_prologue(1)
        for t in range(NT):
            phase_a_tile(1, pa1, t)
            phase_b_tile(0, pa0, t)
        finish_idx(pa1)
        for t in range(NT):
            phase_b_tile(1, pa1, t)

    nc.compile()
    return nc


_NC = None


def kernel(xyzs, feature, ln_g, ln_b, w_qkv, w_sp, w_out, b_out):
    global _NC
    from concourse.bass_utils import run_bass_kernel_spmd
    if _NC is None:
        _NC = _build_nc()
    xyzs = np.asarray(xyzs, np.float32)
    feature = np.asarray(feature, np.float32)
    rep = dict(ln_g=np.asarray(ln_g, np.float32),
               ln_b=np.asarray(ln_b, np.float32),
               w_qkv=np.asarray(w_qkv, np.float32),
               w_sp=np.asarray(w_sp, np.float32),
               w_out=np.asarray(w_out, np.float32),
               b_out=np.asarray(b_out, np.float32))
    in_maps = []
    for c in range(NCORES):
        m = dict(rep)
        m["xyzs"] = xyzs[c * NB:(c + 1) * NB]
        m["feature"] = feature[c * NB:(c + 1) * NB]
        in_maps.append(m)
    res = run_bass_kernel_spmd(_NC, in_maps, list(range(NCORES)))
    out = np.concatenate([res.results[c]["out"] for c in range(NCORES)], axis=0)
    return out.astype(np.float32)


# revision 14
# speedup vs baseline: 1.0068x; 1.0068x over previous
"""Trainium2 Bass kernel for nn_Attention_79070347919638 (gnn_message_passing).

Point-cloud ball-query attention, data-parallel over batch: 16 batches -> 8
NeuronCores x 2 batches each. Per core: LayerNorm+QKV on PE, ball query via a
split-bf16 exact pairwise quadratic-form matmul (16-term hi/lo decomposition,
bf16 operands, fp32 PSUM accumulate) + top-8 smallest-index extraction with
InstMax on an index-encoded value, neighbor gather via one batched indirect
DMA per tile from a DRAM-staged [k|v|xyz] row table, per-point attention on
DVE, output projection + GELU + residual on PE/ACT/DVE.
"""
import sys
import numpy as np

sys.path.insert(0, "/opt/trn_rl_repo")

B, N, D = 16, 2048, 256
H, DH, KNB = 8, 64, 8
I = H * DH  # 512
R2 = 0.09
EPS = 1e-5
NCORES = 8
NB = B // NCORES  # batches per core
P = 128
NT = N // P  # n-tiles per batch
ROW = 1152  # gathered row: k(512) | v(512) | xyz(3) | pad -> 2304B (%256==0 for dma_gather)
BIG_C = 2048.0  # index encoding: val = BIG_C - m for in-radius m
QD = 16  # split-bf16 quadratic form contraction size



def _ap(view, dims):
    """Build an AP from a view's tensor with explicit [step,count] dims."""
    import concourse.bass as bass
    return bass.AP(tensor=view.tensor, offset=view.offset, ap=list(dims))


def _bcast_mid(view3, n):
    """[p, 1, x] view -> [p, n(stride0), x]."""
    return _ap(view3, [view3.ap[0], [0, n], view3.ap[2]])


def _bcast_last(view, n):
    """[p, ...] view -> same dims + [0, n] appended."""
    return _ap(view, list(view.ap) + [[0, n]])


def _build_nc():
    import concourse.bass as bass
    import concourse.bacc as bacc
    import concourse.mybir as mybir
    import concourse.tile as tile
    from concourse.masks import make_identity
    from contextlib import ExitStack

    dt = mybir.dt
    Alu = mybir.AluOpType
    Act = mybir.ActivationFunctionType
    Axis = mybir.AxisListType

    nc = bacc.Bacc("TRN2", target_bir_lowering=False, debug=False,
                   num_devices=NCORES)

    xyzs_d = nc.dram_tensor("xyzs", [NB, N, 3], dt.float32, kind="ExternalInput").ap()
    feat_d = nc.dram_tensor("feature", [NB, N, D], dt.float32, kind="ExternalInput").ap()
    lng_d = nc.dram_tensor("ln_g", [D], dt.float32, kind="ExternalInput").ap()
    lnb_d = nc.dram_tensor("ln_b", [D], dt.float32, kind="ExternalInput").ap()
    wqkv_d = nc.dram_tensor("w_qkv", [D, 3 * I], dt.float32, kind="ExternalInput").ap()
    wsp_d = nc.dram_tensor("w_sp", [3, DH], dt.float32, kind="ExternalInput").ap()
    wout_d = nc.dram_tensor("w_out", [I, D], dt.float32, kind="ExternalInput").ap()
    bout_d = nc.dram_tensor("b_out", [D], dt.float32, kind="ExternalInput").ap()
    out_d = nc.dram_tensor("out", [NB, N, D], dt.float32, kind="ExternalOutput").ap()

    kv_d = [nc.dram_tensor(f"kvrows{b}", [N, ROW], dt.bfloat16).ap()
            for b in range(NB)]

    ctx = ExitStack()
    with tile.TileContext(nc) as tc, ctx:
        cpool = ctx.enter_context(tc.tile_pool(name="const", bufs=1))
        sb = ctx.enter_context(tc.tile_pool(name="sb", bufs=2))
        sb3 = ctx.enter_context(tc.tile_pool(name="sb3", bufs=4))
        w1 = ctx.enter_context(tc.tile_pool(name="w1", bufs=1))
        sb2a = ctx.enter_context(tc.tile_pool(name="sb2a", bufs=4))
        sbg = ctx.enter_context(tc.tile_pool(name="sbg", bufs=2))
        ps_tr = ctx.enter_context(tc.tile_pool(name="ps_tr", bufs=2, space="PSUM"))
        ps_po = ctx.enter_context(tc.tile_pool(name="ps_po", bufs=2, space="PSUM"))
        ps_qkv = ctx.enter_context(tc.tile_pool(name="ps_qkv", bufs=2, space="PSUM"))
        ps_d2 = ctx.enter_context(tc.tile_pool(name="ps_d2", bufs=1, space="PSUM"))

        # ================= one-time constants =================
        ident = cpool.tile([P, P], dt.bfloat16)
        make_identity(nc, ident[:])

        iota_h = cpool.tile([P, N], dt.float16)
        nc.gpsimd.iota(iota_h[:], pattern=[[-1, N]], base=int(BIG_C),
                       channel_multiplier=0,
                       allow_small_or_imprecise_dtypes=True)

        # ln_g-scaled w_qkv (bf16), two K-chunks along free: [128, 2*1536]
        wq_sb = cpool.tile([P, 2 * 3 * I], dt.bfloat16)
        g_col = cpool.tile([P, 2], dt.float32)
        nc.sync.dma_start(g_col[:], lng_d.rearrange("(c p) -> p c", p=P))
        for c in range(2):
            wtmp = w1.tile([P, 3 * I], dt.float32, tag="wtmp")
            nc.sync.dma_start(wtmp[:], wqkv_d[c * P:(c + 1) * P, :])
            nc.vector.tensor_scalar_mul(wq_sb[:, c * 3 * I:(c + 1) * 3 * I],
                                        wtmp[:], g_col[:, c:c + 1])
        # bw = ln_b @ w_qkv  [1, 1536]
        b_col = cpool.tile([P, 2], dt.float32)
        nc.sync.dma_start(b_col[:], lnb_d.rearrange("(c p) -> p c", p=P))
        b_colb = cpool.tile([P, 2], dt.bfloat16)
        nc.vector.tensor_copy(b_colb[:], b_col[:])
        bw_rowb = cpool.tile([1, 3 * I], dt.bfloat16)
        for ch in range(3):
            bw_ps = ps_tr.tile([1, I], dt.float32, tag="ptr")
            for c in range(2):
                nc.tensor.matmul(bw_ps[:1, :], lhsT=b_colb[:, c:c + 1],
                                 rhs=wq_sb[:, c * 3 * I + ch * I:
                                           c * 3 * I + (ch + 1) * I],
                                 start=(c == 0), stop=(c == 1))
            nc.scalar.copy(bw_rowb[:1, ch * I:(ch + 1) * I], bw_ps[:1, :])
        ones1 = cpool.tile([1, P], dt.bfloat16)
        nc.vector.memset(ones1[:1, :], 1.0)

        wout_sb = cpool.tile([P, 4 * D], dt.bfloat16)
        for c in range(4):
            wotmp = w1.tile([P, D], dt.float32, tag="wotmp")
            nc.sync.dma_start(wotmp[:], wout_d[c * P:(c + 1) * P, :])
            nc.vector.tensor_copy(wout_sb[:, c * D:(c + 1) * D], wotmp[:])
        bout_row = cpool.tile([1, D], dt.bfloat16)
        btmp = cpool.tile([1, D], dt.float32)
        nc.sync.dma_start(btmp[:1, :], bout_d[None, :])
        nc.vector.tensor_copy(bout_row[:1, :], btmp[:1, :])

        # WSPOUT = blockdiag_h(w_sp) @ w_out : [24, 256] bf16
        identf = cpool.tile([P, P], dt.float32)
        make_identity(nc, identf[:])
        wsp_sb = cpool.tile([3, DH], dt.float32)
        nc.sync.dma_start(wsp_sb[:3, :], wsp_d[:, :])
        wspT_ps = ps_tr.tile([DH, 3], dt.float32, tag="ptr")
        nc.tensor.transpose(wspT_ps[:DH, :3], wsp_sb[:3, :], identf[:3, :3])
        wspT = cpool.tile([DH, 3], dt.bfloat16)
        nc.scalar.copy(wspT[:DH, :3], wspT_ps[:DH, :3])
        wspbd = cpool.tile([P, 4 * 24], dt.bfloat16)
        nc.vector.memset(wspbd[:], 0.0)
        for h in range(H):
            c, off = divmod(h * DH, P)
            nc.vector.tensor_copy(
                wspbd[off:off + DH, c * 24 + h * 3:c * 24 + h * 3 + 3],
                wspT[:DH, :3])
        wspout_ps = ps_tr.tile([24, D], dt.float32, tag="ptr")
        for c in range(4):
            nc.tensor.matmul(wspout_ps[:24, :], lhsT=wspbd[:, c * 24:(c + 1) * 24],
                             rhs=wout_sb[:, c * D:(c + 1) * D],
                             start=(c == 0), stop=(c == 3))
        wspout = cpool.tile([24, D], dt.bfloat16)
        nc.scalar.copy(wspout[:24, :], wspout_ps[:24, :])

        # ---- constants for the wrapped int16 gather-index build ----
        # h16t[r, p] = (p % 16 == r); g16t[g, p] = (p // 16 == g)
        h16t = cpool.tile([16, P], dt.float32)
        nc.gpsimd.iota(h16t[:16, :], pattern=[[0, 8], [-1, 16]], base=16,
                       channel_multiplier=1,
                       allow_small_or_imprecise_dtypes=True)
        nc.vector.tensor_scalar(h16t[:16, :], h16t[:16, :], 16.0, None,
                                op0=Alu.is_equal)
        g16t = cpool.tile([8, P], dt.float32)
        nc.gpsimd.iota(g16t[:8, :], pattern=[[-1, 8], [0, 16]], base=8,
                       channel_multiplier=1,
                       allow_small_or_imprecise_dtypes=True)
        nc.vector.tensor_scalar(g16t[:8, :], g16t[:8, :], 8.0, None,
                                op0=Alu.is_equal)
        # s128[p, j] = (p % 16 == j % 16) = h16 @ h16t
        s128_ps = ps_tr.tile([P, P], dt.float32, tag="ptr")
        nc.tensor.matmul(s128_ps[:], lhsT=h16t[:16, :], rhs=h16t[:16, :],
                         start=True, stop=True)
        s128 = cpool.tile([P, P], dt.float32)
        nc.scalar.copy(s128[:], s128_ps[:])
        # g16[p, g] = (p // 16 == g)
        g16_ps = ps_tr.tile([P, 8], dt.float32, tag="ptr")
        nc.tensor.transpose(g16_ps[:, :8], g16t[:8, :], identf[:8, :8])
        g16 = cpool.tile([P, 8], dt.float32)
        nc.scalar.copy(g16[:], g16_ps[:, :8])

        # ========== phase A / phase B as per-tile emitters ==========
        def phase_a_prologue(b):
            xyz_t = sb.tile([P, NT * 3], dt.float32, tag="xyz")
            nc.sync.dma_start(
                xyz_t[:].rearrange("p (t c) -> p t c", c=3),
                xyzs_d[b].rearrange("(t p) c -> p t c", p=P))
            sq = sb.tile([P, NT * 3], dt.float32, tag="sq")
            nc.vector.tensor_mul(sq[:], xyz_t[:], xyz_t[:])
            x2 = sb.tile([P, NT], dt.float32, tag="x2")
            nc.vector.tensor_reduce(
                x2[:], sq[:].rearrange("p (t c) -> p t c", c=3),
                axis=Axis.X, op=Alu.add)

            # ---- split-bf16 components ----
            s_f = sb.tile([P, NT], dt.float32, tag="s_f")
            nc.vector.tensor_scalar(s_f[:], x2[:], -1.0, float(R2),
                                    op0=Alu.mult, op1=Alu.add)
            sh_b = sb.tile([P, NT], dt.bfloat16, tag="sh_b")
            nc.vector.tensor_copy(sh_b[:], s_f[:])
            sh_f = sb.tile([P, NT], dt.float32, tag="sh_f")
            nc.vector.tensor_copy(sh_f[:], sh_b[:])
            sl_b = sb.tile([P, NT], dt.bfloat16, tag="sl_b")
            nc.vector.tensor_sub(s_f[:], s_f[:], sh_f[:])
            nc.vector.tensor_copy(sl_b[:], s_f[:])
            ny2 = sb.tile([P, NT], dt.float32, tag="ny2")
            nc.vector.tensor_scalar_mul(ny2[:], x2[:], -1.0)
            y2h_b = sb.tile([P, NT], dt.bfloat16, tag="y2h_b")
            nc.vector.tensor_copy(y2h_b[:], ny2[:])
            y2h_f = sb.tile([P, NT], dt.float32, tag="y2h_f")
            nc.vector.tensor_copy(y2h_f[:], y2h_b[:])
            y2l_b = sb.tile([P, NT], dt.bfloat16, tag="y2l_b")
            nc.vector.tensor_sub(ny2[:], ny2[:], y2h_f[:])
            nc.vector.tensor_copy(y2l_b[:], ny2[:])
            xh_b = sb.tile([P, NT * 3], dt.bfloat16, tag="xh_b")
            nc.vector.tensor_copy(xh_b[:], xyz_t[:])
            xh_f = sb.tile([P, NT * 3], dt.float32, tag="xh_f")
            nc.vector.tensor_copy(xh_f[:], xh_b[:])
            xl_f = sb.tile([P, NT * 3], dt.float32, tag="xl_f")
            nc.vector.tensor_sub(xl_f[:], xyz_t[:], xh_f[:])
            xl_b = sb.tile([P, NT * 3], dt.bfloat16, tag="xl_b")
            nc.vector.tensor_copy(xl_b[:], xl_f[:])

            palla = sb.tile([P, NT * QD], dt.bfloat16, tag="palla")
            pallb = sb.tile([P, NT * QD], dt.bfloat16, tag="pallb")
            pva = palla[:].rearrange("p (t q) -> p t q", q=QD)
            pvb = pallb[:].rearrange("p (t q) -> p t q", q=QD)
            xhv = xh_b[:].rearrange("p (t c) -> p t c", c=3)
            xlv = xl_b[:].rearrange("p (t c) -> p t c", c=3)
            nc.vector.tensor_copy(pva[:, :, 0], sh_b[:])
            nc.vector.tensor_copy(pva[:, :, 1], sl_b[:])
            nc.vector.memset(pva[:, :, 2:4], 1.0)
            for c in range(3):
                nc.vector.tensor_scalar_mul(pva[:, :, 4 + 4 * c], xhv[:, :, c], 2.0)
                nc.vector.tensor_copy(pva[:, :, 5 + 4 * c], pva[:, :, 4 + 4 * c])
                nc.vector.tensor_scalar_mul(pva[:, :, 6 + 4 * c], xlv[:, :, c], 2.0)
                nc.vector.tensor_copy(pva[:, :, 7 + 4 * c], pva[:, :, 6 + 4 * c])
            nc.vector.memset(pvb[:, :, 0:2], 1.0)
            nc.vector.tensor_copy(pvb[:, :, 2], y2h_b[:])
            nc.vector.tensor_copy(pvb[:, :, 3], y2l_b[:])
            for c in range(3):
                nc.vector.tensor_copy(pvb[:, :, 4 + 4 * c], xhv[:, :, c])
                nc.vector.tensor_copy(pvb[:, :, 5 + 4 * c], xlv[:, :, c])
                nc.vector.tensor_copy(pvb[:, :, 6 + 4 * c], xhv[:, :, c])
                nc.vector.tensor_copy(pvb[:, :, 7 + 4 * c], xlv[:, :, c])

            a4 = sb.tile([P, N], dt.bfloat16, tag="a4")
            b4 = sb.tile([P, N], dt.bfloat16, tag="b4")
            for t in range(NT):
                s_ = slice(t * P, (t + 1) * P)
                for (pt, dst) in ((palla, a4), (pallb, b4)):
                    trp16 = ps_tr.tile([QD, P], dt.bfloat16, tag="ptr")
                    nc.tensor.transpose(trp16[:QD, :],
                                        pt[:, t * QD:(t + 1) * QD], ident[:])
                    nc.scalar.copy(dst[0:QD, s_], trp16[:QD, :])
            for st in (32, 64, 96):
                nc.vector.tensor_copy(a4[st:st + QD, :], a4[0:QD, :])
                nc.vector.tensor_copy(b4[st:st + QD, :], b4[0:QD, :])

            q_sb = sb.tile([P, NT * I], dt.bfloat16, tag="q_sb")
            spread = sb.tile([P, NT * 64], dt.float32, tag="spread")
            idx16w = sb.tile([P, NT * 64], dt.int16, tag="idx16w")
            return dict(xyz_t=xyz_t, a4=a4, b4=b4, q_sb=q_sb,
                        spread=spread, idx16w=idx16w)

        def phase_a_tile(b, pa, t):
            xyz_t = pa["xyz_t"]; a4 = pa["a4"]; b4 = pa["b4"]
            q_sb = pa["q_sb"]; spread = pa["spread"]
            ftile = sb3.tile([P, D], dt.float32, tag="ftile")
            nc.sync.dma_start(ftile[:], feat_d[b, t * P:(t + 1) * P, :])
            mean = sb3.tile([P, 1], dt.float32, tag="mean")
            nc.vector.tensor_reduce(mean[:], ftile[:], axis=Axis.X, op=Alu.add)
            nc.vector.tensor_scalar_mul(mean[:], mean[:], 1.0 / D)
            var = sb3.tile([P, 1], dt.float32, tag="var")
            sttd = sb3.tile([P, D], dt.float32, tag="ftile")
            nc.vector.scalar_tensor_tensor(
                sttd[:], in0=ftile[:], scalar=mean[:, :1], in1=ftile[:],
                op0=Alu.subtract, op1=Alu.mult, accum_out=var[:, :1])
            rstd = sb3.tile([P, 1], dt.float32, tag="rstd")
            nc.vector.tensor_scalar(rstd[:], var[:], 1.0 / D, EPS,
                                    op0=Alu.mult, op1=Alu.add)
            nc.vector.reciprocal(rstd[:], rstd[:])
            nc.scalar.sqrt(rstd[:], rstd[:])
            zn = sb3.tile([P, D], dt.bfloat16, tag="zn")
            nc.vector.tensor_scalar(zn[:], ftile[:], mean[:, :1], rstd[:, :1],
                                    op0=Alu.subtract, op1=Alu.mult)
            znT = sb3.tile([P, 2 * P], dt.bfloat16, tag="znT")
            znT_ps = ps_tr.tile([P, 2 * P], dt.bfloat16, tag="ptr")
            for c in range(2):
                nc.tensor.transpose(znT_ps[:, c * P:(c + 1) * P],
                                    zn[:, c * P:(c + 1) * P], ident[:])
            nc.scalar.copy(znT[:], znT_ps[:])
            kv_sb = sb3.tile([P, ROW], dt.bfloat16, tag="kv_sb")
            for ch in range(3):
                qkv_ps = ps_qkv.tile([P, I], dt.float32, tag="qkv")
                for c in range(2):
                    nc.tensor.matmul(
                        qkv_ps[:], lhsT=znT[:, c * P:(c + 1) * P],
                        rhs=wq_sb[:, c * 3 * I + ch * I:
                                  c * 3 * I + (ch + 1) * I],
                        start=(c == 0), stop=False)
                nc.tensor.matmul(
                    qkv_ps[:], lhsT=ones1[:1, :],
                    rhs=bw_rowb[:1, ch * I:(ch + 1) * I],
                    start=False, stop=True)
                if ch == 0:
                    nc.scalar.copy(q_sb[:, t * I:(t + 1) * I], qkv_ps[:])
                else:
                    nc.scalar.copy(kv_sb[:, (ch - 1) * I:ch * I], qkv_ps[:])
            nc.vector.tensor_copy(kv_sb[:, 2 * I:2 * I + 3],
                                  xyz_t[:, t * 3:(t + 1) * 3])
            nc.sync.dma_start(kv_d[b][t * P:(t + 1) * P, :], kv_sb[:])

            # ball query for this tile: d2aug via split-bf16 matmul
            sgn = sb.tile([P, N], dt.float16, tag="sgn")
            for half in range(2):
                d2ps = ps_d2.tile([P, N // 2], dt.float32, tag="d2")
                for j in range(2):
                    mi = half * 2 + j
                    st = 32 * mi
                    nc.tensor.matmul(
                        d2ps[:, j * 512:(j + 1) * 512],
                        lhsT=a4[st:st + QD, t * P:(t + 1) * P],
                        rhs=b4[st:st + QD, mi * 512:(mi + 1) * 512],
                        start=True, stop=True,
                        tile_position=(st, 0))
                nc.scalar.sign(sgn[:, half * (N // 2):(half + 1) * (N // 2)],
                               d2ps[:])
            nc.vector.tensor_mul(sgn[:], sgn[:], iota_h[:])
            v8 = sb.tile([P, 8], dt.float16, tag="v8")
            nc.vector.max(out=v8[:], in_=sgn[:])
            idxf = sb.tile([P, 8], dt.float32, tag="idxf")
            nc.vector.tensor_scalar(idxf[:], v8[:], -1.0, float(BIG_C),
                                    op0=Alu.mult, op1=Alu.add)
            pred = sb.tile([P, 8], dt.uint8, tag="pred")
            nc.vector.tensor_scalar(pred[:], v8[:], 0.0, None, op0=Alu.is_gt)
            idxf2 = sb.tile([P, 8], dt.float32, tag="idxf2")
            nc.vector.select(idxf2[:], pred[:], idxf[:],
                             _ap(idxf[:, 0:1], [idxf[:, 0:1].ap[0], [0, 8]]))
            # spread[p, (k, pg)] = idxf2[p, k] * (p // 16 == pg)
            sp = spread[:, t * 64:(t + 1) * 64]
            nc.vector.tensor_mul(
                sp.rearrange("p (k g) -> p k g", k=8),
                _bcast_last(idxf2[:], 8),
                _ap(g16[:], [g16[:].ap[0], [0, 8], [1, 8]]))

        def phase_b_tile(b, pa, t):
            xyz_t = pa["xyz_t"]; q_sb = pa["q_sb"]; idx16w = pa["idx16w"]
            kvg = sbg.tile([P, 8 * ROW], dt.bfloat16, tag="kvg")
            nc.gpsimd.dma_gather(
                out_ap=kvg[:].rearrange("p (k r) -> p k r", k=8),
                in_ap=kv_d[b][:, :],
                idxs_ap=idx16w[:, t * 64:(t + 1) * 64],
                num_idxs=1024,
                num_idxs_reg=1024,
                elem_size=ROW,
                queue_num=0,
            )
            kview = kvg[:].rearrange("p (k r) -> p k r", k=8)
            qv = q_sb[:, t * I:(t + 1) * I].rearrange("p (o i) -> p o i", o=1)
            wq = sb2a.tile([P, 8 * I], dt.bfloat16, tag="wq")
            nc.vector.tensor_mul(
                wq[:].rearrange("p (k i) -> p k i", k=8),
                kview[:, :, 0:I], _bcast_mid(qv, 8))
            width = DH
            while width > 1:
                half = width // 2
                a = wq[:].rearrange("p (kh w) -> p kh w", w=DH)
                nc.vector.tensor_add(
                    a[:, :, 0:half], a[:, :, 0:half], a[:, :, half:width])
                width = half
            wexp = sb3.tile([P, 64], dt.bfloat16, tag="wexp")
            nc.scalar.activation(
                wexp[:],
                _ap(wq[:], [wq[:].ap[0], [DH, 64]]),
                Act.Exp, scale=float(DH ** -0.5))
            z = sb3.tile([P, 8], dt.float32, tag="z")
            nc.vector.tensor_reduce(
                z[:],
                _ap(wexp[:], [wexp[:].ap[0], [1, 8], [8, 8]]),
                axis=Axis.X, op=Alu.add)
            nc.vector.reciprocal(z[:], z[:])
            zb = sb3.tile([P, 8], dt.bfloat16, tag="zb")
            nc.vector.tensor_copy(zb[:], z[:])
            attn = sb3.tile([P, 64], dt.bfloat16, tag="attn")  # [k,h]
            we2 = wexp[:].rearrange("p (k h) -> p k h", k=8)
            zv = zb[:].rearrange("p (o h) -> p o h", o=1)
            nc.vector.tensor_mul(
                attn[:].rearrange("p (k h) -> p k h", k=8),
                we2, _bcast_mid(zv, 8))
            att2 = sb3.tile([P, P], dt.bfloat16, tag="att2")
            nc.vector.tensor_copy(
                att2[:].rearrange("p (j e) -> p j e", e=2),
                _bcast_last(attn[:].rearrange("p (o j) -> p o j", o=1)[:, 0, :], 2))
            wv = sb2a.tile([P, 8 * I], dt.bfloat16, tag="wq")
            a2 = att2[:]
            nc.vector.tensor_mul(
                _ap(wv[:], [wv[:].ap[0], [512, 8], [64, 8], [2, 32], [1, 2]]),
                _ap(kvg[:, I:I + 1],
                    [kvg[:].ap[0], [ROW, 8], [64, 8], [2, 32], [1, 2]]),
                _ap(a2, [a2.ap[0], [16, 8], [2, 8], [0, 32], [1, 2]]))
            wv2 = wv[:].rearrange("p (k i) -> p k i", k=8)
            nc.vector.tensor_add(wv2[:, 0:4, :], wv2[:, 0:4, :], wv2[:, 4:8, :])
            nc.vector.tensor_add(wv2[:, 0:2, :], wv2[:, 0:2, :], wv2[:, 2:4, :])
            ao = sb3.tile([P, I], dt.bfloat16, tag="ao")
            nc.vector.tensor_add(ao[:].rearrange("p (o i) -> p o i", o=1),
                                 wv2[:, 0:1, :], wv2[:, 1:2, :])
            disp = sb3.tile([P, 24], dt.bfloat16, tag="disp")  # [k,c]
            xv = xyz_t[:, t * 3:(t + 1) * 3].rearrange("p (o c) -> p o c", o=1)
            nc.vector.tensor_sub(
                disp[:].rearrange("p (k c) -> p k c", k=8),
                kview[:, :, 2 * I:2 * I + 3], _bcast_mid(xv, 8))
            dprod = sb3.tile([P, H * 8 * 3], dt.bfloat16, tag="dprod")
            dp3 = dprod[:].rearrange("p (h k c) -> p h k c", h=H, k=8)
            dview = disp[:].rearrange("p (k c) -> p k c", k=8)
            ahk = attn[:].rearrange("p (k h) -> p h k", k=8)
            nc.vector.tensor_mul(
                dp3,
                _ap(dview, [dview.ap[0], [0, H], dview.ap[1], dview.ap[2]]),
                _bcast_last(ahk, 3))
            TT = nc.vector.tensor_tensor
            TT(dp3[:, :, 0:4, :], dp3[:, :, 0:4, :], dp3[:, :, 4:8, :],
               op=Alu.max)
            TT(dp3[:, :, 0:2, :], dp3[:, :, 0:2, :], dp3[:, :, 2:4, :],
               op=Alu.max)
            dis = sb3.tile([P, 24], dt.bfloat16, tag="dis")  # [h,c]
            TT(dis[:].rearrange("p (h o c) -> p h o c", o=1, c=3),
               dp3[:, :, 0:1, :], dp3[:, :, 1:2, :], op=Alu.max)
            aot = sb3.tile([P, 4 * P], dt.bfloat16, tag="aot")
            aot_ps = ps_tr.tile([P, 4 * P], dt.bfloat16, tag="ptr")
            for c in range(4):
                nc.tensor.transpose(aot_ps[:, c * P:(c + 1) * P],
                                    ao[:, c * P:(c + 1) * P], ident[:])
            nc.scalar.copy(aot[:], aot_ps[:])
            dist = sb3.tile([24, P], dt.bfloat16, tag="dist")
            trp = ps_tr.tile([P, P], dt.bfloat16, tag="ptr")
            nc.tensor.transpose(trp[:24, :], dis[:, :24], ident[:])
            nc.scalar.copy(dist[:24, :], trp[:24, :])
            po = ps_po.tile([P, D], dt.float32, tag="po")
            for c in range(4):
                nc.tensor.matmul(po[:], lhsT=aot[:, c * P:(c + 1) * P],
                                 rhs=wout_sb[:, c * D:(c + 1) * D],
                                 start=(c == 0), stop=False)
            nc.tensor.matmul(po[:], lhsT=dist[:24, :], rhs=wspout[:24, :],
                             start=False, stop=False)
            nc.tensor.matmul(po[:], lhsT=ones1[:1, :], rhs=bout_row[:1, :],
                             start=False, stop=True)
            gel = sb3.tile([P, D], dt.float32, tag="gel")
            nc.scalar.activation(gel[:], po[:], Act.Gelu)
            f2 = sb3.tile([P, D], dt.float32, tag="ftile")
            nc.sync.dma_start(f2[:], feat_d[b, t * P:(t + 1) * P, :])
            outt = sb3.tile([P, D], dt.float32, tag="outt")
            nc.vector.tensor_add(outt[:], gel[:], f2[:])
            nc.sync.dma_start(out_d[b, t * P:(t + 1) * P, :], outt[:])

        # ===== schedule: A(b0) | A(b1)+B(b0) interleaved | B(b1) =====
        def finish_idx(pa):
            # idx16w[p, m (global)] = sum_q s128[q, p] * spread[q, m]
            for c2 in range(2):
                wps = ps_tr.tile([P, 512], dt.float32, tag="ptr")
                nc.tensor.matmul(wps[:], lhsT=s128[:],
                                 rhs=pa["spread"][:, c2 * 512:(c2 + 1) * 512],
                                 start=True, stop=True)
                nc.vector.tensor_copy(pa["idx16w"][:, c2 * 512:(c2 + 1) * 512],
                                      wps[:])

        pa0 = phase_a_prologue(0)
        for t in range(NT):
            phase_a_tile(0, pa0, t)
        finish_idx(pa0)
        pa1 = phase_a_prologue(1)
        for t in range(NT):
            phase_a_tile(1, pa1, t)
            phase_b_tile(0, pa0, t)
        finish_idx(pa1)
        for t in range(NT):
            phase_b_tile(1, pa1, t)

    nc.compile()
    return nc


_NC = None


def kernel(xyzs, feature, ln_g, ln_b, w_qkv, w_sp, w_out, b_out):
    global _NC
    from concourse.bass_utils import run_bass_kernel_spmd
    if _NC is None:
        _NC = _build_nc()
    xyzs = np.asarray(xyzs, np.float32)
    feature = np.asarray(feature, np.float32)
    rep = dict(ln_g=np.asarray(ln_g, np.float32),
               ln_b=np.asarray(ln_b, np.float32),
               w_qkv=np.asarray(w_qkv, np.float32),
               w_sp=np.asarray(w_sp, np.float32),
               w_out=np.asarray(w_out, np.float32),
               b_out=np.asarray(b_out, np.float32))
    in_maps = []
    for c in range(NCORES):
        m = dict(rep)
        m["xyzs"] = xyzs[c * NB:(c + 1) * NB]
        m["feature"] = feature[c * NB:(c + 1) * NB]
        in_maps.append(m)
    res = run_bass_kernel_spmd(_NC, in_maps, list(range(NCORES)))
    out = np.concatenate([res.results[c]["out"] for c in range(NCORES)], axis=0)
    return out.astype(np.float32)


# revision 15
# speedup vs baseline: 1.0273x; 1.0204x over previous
"""Trainium2 Bass kernel for nn_Attention_79070347919638 (gnn_message_passing).

Point-cloud ball-query attention, data-parallel over batch: 16 batches -> 8
NeuronCores x 2 batches each. Per core: LayerNorm+QKV on PE, ball query via a
split-bf16 exact pairwise quadratic-form matmul (16-term hi/lo decomposition,
bf16 operands, fp32 PSUM accumulate) + top-8 smallest-index extraction with
InstMax on an index-encoded value, neighbor gather via one batched indirect
DMA per tile from a DRAM-staged [k|v|xyz] row table, per-point attention on
DVE, output projection + GELU + residual on PE/ACT/DVE.
"""
import sys
import numpy as np

sys.path.insert(0, "/opt/trn_rl_repo")

B, N, D = 16, 2048, 256
H, DH, KNB = 8, 64, 8
I = H * DH  # 512
R2 = 0.09
EPS = 1e-5
NCORES = 8
NB = B // NCORES  # batches per core
P = 128
NT = N // P  # n-tiles per batch
ROW = 1040  # gathered row: k(512) | v(512) | xyz(3) | pad -> 32B aligned
BIG_C = 2048.0  # index encoding: val = BIG_C - m for in-radius m
QD = 16  # split-bf16 quadratic form contraction size



def _ap(view, dims):
    """Build an AP from a view's tensor with explicit [step,count] dims."""
    import concourse.bass as bass
    return bass.AP(tensor=view.tensor, offset=view.offset, ap=list(dims))


def _bcast_mid(view3, n):
    """[p, 1, x] view -> [p, n(stride0), x]."""
    return _ap(view3, [view3.ap[0], [0, n], view3.ap[2]])


def _bcast_last(view, n):
    """[p, ...] view -> same dims + [0, n] appended."""
    return _ap(view, list(view.ap) + [[0, n]])


def _build_nc():
    import concourse.bass as bass
    import concourse.bacc as bacc
    import concourse.mybir as mybir
    import concourse.tile as tile
    from concourse.masks import make_identity
    from contextlib import ExitStack

    dt = mybir.dt
    Alu = mybir.AluOpType
    Act = mybir.ActivationFunctionType
    Axis = mybir.AxisListType

    nc = bacc.Bacc("TRN2", target_bir_lowering=False, debug=False,
                   num_devices=NCORES)

    xyzs_d = nc.dram_tensor("xyzs", [NB, N, 3], dt.float32, kind="ExternalInput").ap()
    feat_d = nc.dram_tensor("feature", [NB, N, D], dt.float32, kind="ExternalInput").ap()
    lng_d = nc.dram_tensor("ln_g", [D], dt.float32, kind="ExternalInput").ap()
    lnb_d = nc.dram_tensor("ln_b", [D], dt.float32, kind="ExternalInput").ap()
    wqkv_d = nc.dram_tensor("w_qkv", [D, 3 * I], dt.float32, kind="ExternalInput").ap()
    wsp_d = nc.dram_tensor("w_sp", [3, DH], dt.float32, kind="ExternalInput").ap()
    wout_d = nc.dram_tensor("w_out", [I, D], dt.float32, kind="ExternalInput").ap()
    bout_d = nc.dram_tensor("b_out", [D], dt.float32, kind="ExternalInput").ap()
    out_d = nc.dram_tensor("out", [NB, N, D], dt.float32, kind="ExternalOutput").ap()

    kv_d = [nc.dram_tensor(f"kvrows{b}", [N, ROW], dt.bfloat16).ap()
            for b in range(NB)]

    ctx = ExitStack()
    with tile.TileContext(nc) as tc, ctx:
        cpool = ctx.enter_context(tc.tile_pool(name="const", bufs=1))
        sb = ctx.enter_context(tc.tile_pool(name="sb", bufs=2))
        sb3 = ctx.enter_context(tc.tile_pool(name="sb3", bufs=4))
        w1 = ctx.enter_context(tc.tile_pool(name="w1", bufs=1))
        sb2a = ctx.enter_context(tc.tile_pool(name="sb2a", bufs=4))
        sbg = ctx.enter_context(tc.tile_pool(name="sbg", bufs=2))
        ps_tr = ctx.enter_context(tc.tile_pool(name="ps_tr", bufs=2, space="PSUM"))
        ps_po = ctx.enter_context(tc.tile_pool(name="ps_po", bufs=2, space="PSUM"))
        ps_qkv = ctx.enter_context(tc.tile_pool(name="ps_qkv", bufs=2, space="PSUM"))
        ps_d2 = ctx.enter_context(tc.tile_pool(name="ps_d2", bufs=1, space="PSUM"))

        # ================= one-time constants =================
        ident = cpool.tile([P, P], dt.bfloat16)
        make_identity(nc, ident[:])

        iota_h = cpool.tile([P, N], dt.float16)
        nc.gpsimd.iota(iota_h[:], pattern=[[-1, N]], base=int(BIG_C),
                       channel_multiplier=0,
                       allow_small_or_imprecise_dtypes=True)

        # ln_g-scaled w_qkv (bf16), two K-chunks along free: [128, 2*1536]
        wq_sb = cpool.tile([P, 2 * 3 * I], dt.bfloat16)
        g_col = cpool.tile([P, 2], dt.float32)
        nc.sync.dma_start(g_col[:], lng_d.rearrange("(c p) -> p c", p=P))
        for c in range(2):
            wtmp = w1.tile([P, 3 * I], dt.float32, tag="wtmp")
            nc.sync.dma_start(wtmp[:], wqkv_d[c * P:(c + 1) * P, :])
            nc.vector.tensor_scalar_mul(wq_sb[:, c * 3 * I:(c + 1) * 3 * I],
                                        wtmp[:], g_col[:, c:c + 1])
        # bw = ln_b @ w_qkv  [1, 1536]
        b_col = cpool.tile([P, 2], dt.float32)
        nc.sync.dma_start(b_col[:], lnb_d.rearrange("(c p) -> p c", p=P))
        b_colb = cpool.tile([P, 2], dt.bfloat16)
        nc.vector.tensor_copy(b_colb[:], b_col[:])
        bw_rowb = cpool.tile([1, 3 * I], dt.bfloat16)
        for ch in range(3):
            bw_ps = ps_tr.tile([1, I], dt.float32, tag="ptr")
            for c in range(2):
                nc.tensor.matmul(bw_ps[:1, :], lhsT=b_colb[:, c:c + 1],
                                 rhs=wq_sb[:, c * 3 * I + ch * I:
                                           c * 3 * I + (ch + 1) * I],
                                 start=(c == 0), stop=(c == 1))
            nc.scalar.copy(bw_rowb[:1, ch * I:(ch + 1) * I], bw_ps[:1, :])
        ones1 = cpool.tile([1, P], dt.bfloat16)
        nc.vector.memset(ones1[:1, :], 1.0)

        wout_sb = cpool.tile([P, 4 * D], dt.bfloat16)
        for c in range(4):
            wotmp = w1.tile([P, D], dt.float32, tag="wotmp")
            nc.sync.dma_start(wotmp[:], wout_d[c * P:(c + 1) * P, :])
            nc.vector.tensor_copy(wout_sb[:, c * D:(c + 1) * D], wotmp[:])
        bout_row = cpool.tile([1, D], dt.bfloat16)
        btmp = cpool.tile([1, D], dt.float32)
        nc.sync.dma_start(btmp[:1, :], bout_d[None, :])
        nc.vector.tensor_copy(bout_row[:1, :], btmp[:1, :])

        # WSPOUT = blockdiag_h(w_sp) @ w_out : [24, 256] bf16
        identf = cpool.tile([P, P], dt.float32)
        make_identity(nc, identf[:])
        wsp_sb = cpool.tile([3, DH], dt.float32)
        nc.sync.dma_start(wsp_sb[:3, :], wsp_d[:, :])
        wspT_ps = ps_tr.tile([DH, 3], dt.float32, tag="ptr")
        nc.tensor.transpose(wspT_ps[:DH, :3], wsp_sb[:3, :], identf[:3, :3])
        wspT = cpool.tile([DH, 3], dt.bfloat16)
        nc.scalar.copy(wspT[:DH, :3], wspT_ps[:DH, :3])
        wspbd = cpool.tile([P, 4 * 24], dt.bfloat16)
        nc.vector.memset(wspbd[:], 0.0)
        for h in range(H):
            c, off = divmod(h * DH, P)
            nc.vector.tensor_copy(
                wspbd[off:off + DH, c * 24 + h * 3:c * 24 + h * 3 + 3],
                wspT[:DH, :3])
        wspout_ps = ps_tr.tile([24, D], dt.float32, tag="ptr")
        for c in range(4):
            nc.tensor.matmul(wspout_ps[:24, :], lhsT=wspbd[:, c * 24:(c + 1) * 24],
                             rhs=wout_sb[:, c * D:(c + 1) * D],
                             start=(c == 0), stop=(c == 3))
        wspout = cpool.tile([24, D], dt.bfloat16)
        nc.scalar.copy(wspout[:24, :], wspout_ps[:24, :])


        # ========== phase A / phase B as per-tile emitters ==========
        def phase_a_prologue(b):
            xyz_t = sb.tile([P, NT * 3], dt.float32, tag="xyz")
            nc.sync.dma_start(
                xyz_t[:].rearrange("p (t c) -> p t c", c=3),
                xyzs_d[b].rearrange("(t p) c -> p t c", p=P))
            sq = sb.tile([P, NT * 3], dt.float32, tag="sq")
            nc.vector.tensor_mul(sq[:], xyz_t[:], xyz_t[:])
            x2 = sb.tile([P, NT], dt.float32, tag="x2")
            nc.vector.tensor_reduce(
                x2[:], sq[:].rearrange("p (t c) -> p t c", c=3),
                axis=Axis.X, op=Alu.add)

            # ---- split-bf16 components ----
            s_f = sb.tile([P, NT], dt.float32, tag="s_f")
            nc.vector.tensor_scalar(s_f[:], x2[:], -1.0, float(R2),
                                    op0=Alu.mult, op1=Alu.add)
            sh_b = sb.tile([P, NT], dt.bfloat16, tag="sh_b")
            nc.vector.tensor_copy(sh_b[:], s_f[:])
            sh_f = sb.tile([P, NT], dt.float32, tag="sh_f")
            nc.vector.tensor_copy(sh_f[:], sh_b[:])
            sl_b = sb.tile([P, NT], dt.bfloat16, tag="sl_b")
            nc.vector.tensor_sub(s_f[:], s_f[:], sh_f[:])
            nc.vector.tensor_copy(sl_b[:], s_f[:])
            ny2 = sb.tile([P, NT], dt.float32, tag="ny2")
            nc.vector.tensor_scalar_mul(ny2[:], x2[:], -1.0)
            y2h_b = sb.tile([P, NT], dt.bfloat16, tag="y2h_b")
            nc.vector.tensor_copy(y2h_b[:], ny2[:])
            y2h_f = sb.tile([P, NT], dt.float32, tag="y2h_f")
            nc.vector.tensor_copy(y2h_f[:], y2h_b[:])
            y2l_b = sb.tile([P, NT], dt.bfloat16, tag="y2l_b")
            nc.vector.tensor_sub(ny2[:], ny2[:], y2h_f[:])
            nc.vector.tensor_copy(y2l_b[:], ny2[:])
            xh_b = sb.tile([P, NT * 3], dt.bfloat16, tag="xh_b")
            nc.vector.tensor_copy(xh_b[:], xyz_t[:])
            xh_f = sb.tile([P, NT * 3], dt.float32, tag="xh_f")
            nc.vector.tensor_copy(xh_f[:], xh_b[:])
            xl_f = sb.tile([P, NT * 3], dt.float32, tag="xl_f")
            nc.vector.tensor_sub(xl_f[:], xyz_t[:], xh_f[:])
            xl_b = sb.tile([P, NT * 3], dt.bfloat16, tag="xl_b")
            nc.vector.tensor_copy(xl_b[:], xl_f[:])

            palla = sb.tile([P, NT * QD], dt.bfloat16, tag="palla")
            pallb = sb.tile([P, NT * QD], dt.bfloat16, tag="pallb")
            pva = palla[:].rearrange("p (t q) -> p t q", q=QD)
            pvb = pallb[:].rearrange("p (t q) -> p t q", q=QD)
            xhv = xh_b[:].rearrange("p (t c) -> p t c", c=3)
            xlv = xl_b[:].rearrange("p (t c) -> p t c", c=3)
            nc.vector.tensor_copy(pva[:, :, 0], sh_b[:])
            nc.vector.tensor_copy(pva[:, :, 1], sl_b[:])
            nc.vector.memset(pva[:, :, 2:4], 1.0)
            for c in range(3):
                nc.vector.tensor_scalar_mul(pva[:, :, 4 + 4 * c], xhv[:, :, c], 2.0)
                nc.vector.tensor_copy(pva[:, :, 5 + 4 * c], pva[:, :, 4 + 4 * c])
                nc.vector.tensor_scalar_mul(pva[:, :, 6 + 4 * c], xlv[:, :, c], 2.0)
                nc.vector.tensor_copy(pva[:, :, 7 + 4 * c], pva[:, :, 6 + 4 * c])
            nc.vector.memset(pvb[:, :, 0:2], 1.0)
            nc.vector.tensor_copy(pvb[:, :, 2], y2h_b[:])
            nc.vector.tensor_copy(pvb[:, :, 3], y2l_b[:])
            for c in range(3):
                nc.vector.tensor_copy(pvb[:, :, 4 + 4 * c], xhv[:, :, c])
                nc.vector.tensor_copy(pvb[:, :, 5 + 4 * c], xlv[:, :, c])
                nc.vector.tensor_copy(pvb[:, :, 6 + 4 * c], xhv[:, :, c])
                nc.vector.tensor_copy(pvb[:, :, 7 + 4 * c], xlv[:, :, c])

            a4 = sb.tile([P, N], dt.bfloat16, tag="a4")
            b4 = sb.tile([P, N], dt.bfloat16, tag="b4")
            for t in range(NT):
                s_ = slice(t * P, (t + 1) * P)
                for (pt, dst) in ((palla, a4), (pallb, b4)):
                    trp16 = ps_tr.tile([QD, P], dt.bfloat16, tag="ptr")
                    nc.tensor.transpose(trp16[:QD, :],
                                        pt[:, t * QD:(t + 1) * QD], ident[:])
                    nc.scalar.copy(dst[0:QD, s_], trp16[:QD, :])
            for st in (32, 64, 96):
                nc.vector.tensor_copy(a4[st:st + QD, :], a4[0:QD, :])
                nc.vector.tensor_copy(b4[st:st + QD, :], b4[0:QD, :])

            q_sb = sb.tile([P, NT * I], dt.bfloat16, tag="q_sb")
            idx32 = sb.tile([P, NT * 8], dt.int32, tag="idx32")
            return dict(xyz_t=xyz_t, a4=a4, b4=b4, q_sb=q_sb, idx32=idx32)

        def phase_a_tile(b, pa, t):
            xyz_t = pa["xyz_t"]; a4 = pa["a4"]; b4 = pa["b4"]
            q_sb = pa["q_sb"]; idx32 = pa["idx32"]
            ftile = sb3.tile([P, D], dt.float32, tag="ftile")
            nc.sync.dma_start(ftile[:], feat_d[b, t * P:(t + 1) * P, :])
            mean = sb3.tile([P, 1], dt.float32, tag="mean")
            nc.vector.tensor_reduce(mean[:], ftile[:], axis=Axis.X, op=Alu.add)
            nc.vector.tensor_scalar_mul(mean[:], mean[:], 1.0 / D)
            var = sb3.tile([P, 1], dt.float32, tag="var")
            sttd = sb3.tile([P, D], dt.float32, tag="ftile")
            nc.vector.scalar_tensor_tensor(
                sttd[:], in0=ftile[:], scalar=mean[:, :1], in1=ftile[:],
                op0=Alu.subtract, op1=Alu.mult, accum_out=var[:, :1])
            rstd = sb3.tile([P, 1], dt.float32, tag="rstd")
            nc.vector.tensor_scalar(rstd[:], var[:], 1.0 / D, EPS,
                                    op0=Alu.mult, op1=Alu.add)
            nc.vector.reciprocal(rstd[:], rstd[:])
            nc.scalar.sqrt(rstd[:], rstd[:])
            zn = sb3.tile([P, D], dt.bfloat16, tag="zn")
            nc.vector.tensor_scalar(zn[:], ftile[:], mean[:, :1], rstd[:, :1],
                                    op0=Alu.subtract, op1=Alu.mult)
            znT = sb3.tile([P, 2 * P], dt.bfloat16, tag="znT")
            znT_ps = ps_tr.tile([P, 2 * P], dt.bfloat16, tag="ptr")
            for c in range(2):
                nc.tensor.transpose(znT_ps[:, c * P:(c + 1) * P],
                                    zn[:, c * P:(c + 1) * P], ident[:])
            nc.scalar.copy(znT[:], znT_ps[:])
            kv_sb = sb3.tile([P, ROW], dt.bfloat16, tag="kv_sb")
            for ch in range(3):
                qkv_ps = ps_qkv.tile([P, I], dt.float32, tag="qkv")
                for c in range(2):
                    nc.tensor.matmul(
                        qkv_ps[:], lhsT=znT[:, c * P:(c + 1) * P],
                        rhs=wq_sb[:, c * 3 * I + ch * I:
                                  c * 3 * I + (ch + 1) * I],
                        start=(c == 0), stop=False)
                nc.tensor.matmul(
                    qkv_ps[:], lhsT=ones1[:1, :],
                    rhs=bw_rowb[:1, ch * I:(ch + 1) * I],
                    start=False, stop=True)
                if ch == 0:
                    nc.scalar.copy(q_sb[:, t * I:(t + 1) * I], qkv_ps[:])
                else:
                    nc.scalar.copy(kv_sb[:, (ch - 1) * I:ch * I], qkv_ps[:])
            nc.vector.tensor_copy(kv_sb[:, 2 * I:2 * I + 3],
                                  xyz_t[:, t * 3:(t + 1) * 3])
            nc.vector.memset(kv_sb[:, 2 * I + 3:], 0.0)
            nc.sync.dma_start(kv_d[b][t * P:(t + 1) * P, :], kv_sb[:])

            # ball query for this tile: d2aug via split-bf16 matmul
            sgn = sb.tile([P, N], dt.float16, tag="sgn")
            for half in range(2):
                d2ps = ps_d2.tile([P, N // 2], dt.float32, tag="d2")
                for j in range(2):
                    mi = half * 2 + j
                    st = 32 * mi
                    nc.tensor.matmul(
                        d2ps[:, j * 512:(j + 1) * 512],
                        lhsT=a4[st:st + QD, t * P:(t + 1) * P],
                        rhs=b4[st:st + QD, mi * 512:(mi + 1) * 512],
                        start=True, stop=True,
                        tile_position=(st, 0))
                nc.scalar.sign(sgn[:, half * (N // 2):(half + 1) * (N // 2)],
                               d2ps[:])
            nc.vector.tensor_mul(sgn[:], sgn[:], iota_h[:])
            v8 = sb.tile([P, 8], dt.float16, tag="v8")
            nc.vector.max(out=v8[:], in_=sgn[:])
            idxf = sb.tile([P, 8], dt.float32, tag="idxf")
            nc.vector.tensor_scalar(idxf[:], v8[:], -1.0, float(BIG_C),
                                    op0=Alu.mult, op1=Alu.add)
            pred = sb.tile([P, 8], dt.uint8, tag="pred")
            nc.vector.tensor_scalar(pred[:], v8[:], 0.0, None, op0=Alu.is_gt)
            idxf2 = sb.tile([P, 8], dt.float32, tag="idxf2")
            nc.vector.select(idxf2[:], pred[:], idxf[:],
                             _ap(idxf[:, 0:1], [idxf[:, 0:1].ap[0], [0, 8]]))
            nc.vector.tensor_copy(idx32[:, t * 8:(t + 1) * 8], idxf2[:])

        def phase_b_tile(b, pa, t):
            xyz_t = pa["xyz_t"]; q_sb = pa["q_sb"]; idx32 = pa["idx32"]
            kvg = sbg.tile([P, 8 * ROW], dt.bfloat16, tag="kvg")
            for k in range(KNB):
                nc.gpsimd.indirect_dma_start(
                    out=kvg[:, k * ROW:(k + 1) * ROW],
                    out_offset=None,
                    in_=kv_d[b][:, :],
                    in_offset=bass.IndirectOffsetOnAxis(
                        ap=idx32[:, t * 8 + k:t * 8 + k + 1], axis=0),
                )
            kview = kvg[:].rearrange("p (k r) -> p k r", k=8)
            qv = q_sb[:, t * I:(t + 1) * I].rearrange("p (o i) -> p o i", o=1)
            wq = sb2a.tile([P, 8 * I], dt.bfloat16, tag="wq")
            nc.vector.tensor_mul(
                wq[:].rearrange("p (k i) -> p k i", k=8),
                kview[:, :, 0:I], _bcast_mid(qv, 8))
            width = DH
            while width > 1:
                half = width // 2
                a = wq[:].rearrange("p (kh w) -> p kh w", w=DH)
                nc.vector.tensor_add(
                    a[:, :, 0:half], a[:, :, 0:half], a[:, :, half:width])
                width = half
            wexp = sb3.tile([P, 64], dt.bfloat16, tag="wexp")
            nc.scalar.activation(
                wexp[:],
                _ap(wq[:], [wq[:].ap[0], [DH, 64]]),
                Act.Exp, scale=float(DH ** -0.5))
            z = sb3.tile([P, 8], dt.float32, tag="z")
            nc.vector.tensor_reduce(
                z[:],
                _ap(wexp[:], [wexp[:].ap[0], [1, 8], [8, 8]]),
                axis=Axis.X, op=Alu.add)
            nc.vector.reciprocal(z[:], z[:])
            zb = sb3.tile([P, 8], dt.bfloat16, tag="zb")
            nc.vector.tensor_copy(zb[:], z[:])
            attn = sb3.tile([P, 64], dt.bfloat16, tag="attn")  # [k,h]
            we2 = wexp[:].rearrange("p (k h) -> p k h", k=8)
            zv = zb[:].rearrange("p (o h) -> p o h", o=1)
            nc.vector.tensor_mul(
                attn[:].rearrange("p (k h) -> p k h", k=8),
                we2, _bcast_mid(zv, 8))
            att2 = sb3.tile([P, P], dt.bfloat16, tag="att2")
            nc.vector.tensor_copy(
                att2[:].rearrange("p (j e) -> p j e", e=2),
                _bcast_last(attn[:].rearrange("p (o j) -> p o j", o=1)[:, 0, :], 2))
            wv = sb2a.tile([P, 8 * I], dt.bfloat16, tag="wq")
            a2 = att2[:]
            nc.vector.tensor_mul(
                _ap(wv[:], [wv[:].ap[0], [512, 8], [64, 8], [2, 32], [1, 2]]),
                _ap(kvg[:, I:I + 1],
                    [kvg[:].ap[0], [ROW, 8], [64, 8], [2, 32], [1, 2]]),
                _ap(a2, [a2.ap[0], [16, 8], [2, 8], [0, 32], [1, 2]]))
            wv2 = wv[:].rearrange("p (k i) -> p k i", k=8)
            nc.vector.tensor_add(wv2[:, 0:4, :], wv2[:, 0:4, :], wv2[:, 4:8, :])
            nc.vector.tensor_add(wv2[:, 0:2, :], wv2[:, 0:2, :], wv2[:, 2:4, :])
            ao = sb3.tile([P, I], dt.bfloat16, tag="ao")
            nc.vector.tensor_add(ao[:].rearrange("p (o i) -> p o i", o=1),
                                 wv2[:, 0:1, :], wv2[:, 1:2, :])
            disp = sb3.tile([P, 24], dt.bfloat16, tag="disp")  # [k,c]
            xv = xyz_t[:, t * 3:(t + 1) * 3].rearrange("p (o c) -> p o c", o=1)
            nc.vector.tensor_sub(
                disp[:].rearrange("p (k c) -> p k c", k=8),
                kview[:, :, 2 * I:2 * I + 3], _bcast_mid(xv, 8))
            dprod = sb3.tile([P, H * 8 * 3], dt.bfloat16, tag="dprod")
            dp3 = dprod[:].rearrange("p (h k c) -> p h k c", h=H, k=8)
            dview = disp[:].rearrange("p (k c) -> p k c", k=8)
            ahk = attn[:].rearrange("p (k h) -> p h k", k=8)
            nc.vector.tensor_mul(
                dp3,
                _ap(dview, [dview.ap[0], [0, H], dview.ap[1], dview.ap[2]]),
                _bcast_last(ahk, 3))
            TT = nc.vector.tensor_tensor
            TT(dp3[:, :, 0:4, :], dp3[:, :, 0:4, :], dp3[:, :, 4:8, :],
               op=Alu.max)
            TT(dp3[:, :, 0:2, :], dp3[:, :, 0:2, :], dp3[:, :, 2:4, :],
               op=Alu.max)
            dis = sb3.tile([P, 24], dt.bfloat16, tag="dis")  # [h,c]
            TT(dis[:].rearrange("p (h o c) -> p h o c", o=1, c=3),
               dp3[:, :, 0:1, :], dp3[:, :, 1:2, :], op=Alu.max)
            aot = sb3.tile([P, 4 * P], dt.bfloat16, tag="aot")
            aot_ps = ps_tr.tile([P, 4 * P], dt.bfloat16, tag="ptr")
            for c in range(4):
                nc.tensor.transpose(aot_ps[:, c * P:(c + 1) * P],
                                    ao[:, c * P:(c + 1) * P], ident[:])
            nc.scalar.copy(aot[:], aot_ps[:])
            dist = sb3.tile([24, P], dt.bfloat16, tag="dist")
            trp = ps_tr.tile([P, P], dt.bfloat16, tag="ptr")
            nc.tensor.transpose(trp[:24, :], dis[:, :24], ident[:])
            nc.scalar.copy(dist[:24, :], trp[:24, :])
            po = ps_po.tile([P, D], dt.float32, tag="po")
            for c in range(4):
                nc.tensor.matmul(po[:], lhsT=aot[:, c * P:(c + 1) * P],
                                 rhs=wout_sb[:, c * D:(c + 1) * D],
                                 start=(c == 0), stop=False)
            nc.tensor.matmul(po[:], lhsT=dist[:24, :], rhs=wspout[:24, :],
                             start=False, stop=False)
            nc.tensor.matmul(po[:], lhsT=ones1[:1, :], rhs=bout_row[:1, :],
                             start=False, stop=True)
            gel = sb3.tile([P, D], dt.float32, tag="gel")
            nc.scalar.activation(gel[:], po[:], Act.Gelu)
            f2 = sb3.tile([P, D], dt.float32, tag="ftile")
            nc.sync.dma_start(f2[:], feat_d[b, t * P:(t + 1) * P, :])
            outt = sb3.tile([P, D], dt.float32, tag="outt")
            nc.vector.tensor_add(outt[:], gel[:], f2[:])
            nc.sync.dma_start(out_d[b, t * P:(t + 1) * P, :], outt[:])

        # ===== schedule: A(b0) | A(b1)+B(b0) interleaved | B(b1) =====
        pa0 = phase_a_prologue(0)
        for t in range(NT):
            phase_a_tile(0, pa0, t)
        pa1 = phase_a_prologue(1)
        for t in range(NT):
            phase_a_tile(1, pa1, t)
            phase_b_tile(0, pa0, t)
        for t in range(NT):
            phase_b_tile(1, pa1, t)

    nc.compile()
    return nc


_NC = None


def kernel(xyzs, feature, ln_g, ln_b, w_qkv, w_sp, w_out, b_out):
    global _NC
    from concourse.bass_utils import run_bass_kernel_spmd
    if _NC is None:
        _NC = _build_nc()
    xyzs = np.asarray(xyzs, np.float32)
    feature = np.asarray(feature, np.float32)
    rep = dict(ln_g=np.asarray(ln_g, np.float32),
               ln_b=np.asarray(ln_b, np.float32),
               w_qkv=np.asarray(w_qkv, np.float32),
               w_sp=np.asarray(w_sp, np.float32),
               w_out=np.asarray(w_out, np.float32),
               b_out=np.asarray(b_out, np.float32))
    in_maps = []
    for c in range(NCORES):
        m = dict(rep)
        m["xyzs"] = xyzs[c * NB:(c + 1) * NB]
        m["feature"] = feature[c * NB:(c + 1) * NB]
        in_maps.append(m)
    res = run_bass_kernel_spmd(_NC, in_maps, list(range(NCORES)))
    out = np.concatenate([res.results[c]["out"] for c in range(NCORES)], axis=0)
    return out.astype(np.float32)
